# revision 39
# baseline (speedup 1.0000x reference)
"""MoE MLP (top-2 of 8 experts) on 8 Trainium2 NeuronCores.

Strategy: expert parallelism. Each of the 8 cores owns one expert.
Host-side (inside kernel()): route tokens to experts, gather each
expert's tokens (pre-scaled by their routing weight) into a dense
padded [H, T_cap] activation block (transposed so it is directly
usable as the matmul moving operand), and ship it with that expert's
weights to its core. Each core runs two dense bf16 matmuls
(down -> relu -> up) entirely out of SBUF. Host-side combine is a
pure gather-add: every token has exactly K=2 expert contributions.

Device compute per core (bf16, fp32 PSUM accumulation):
  hidT[D, T] = down[H, D]^T @ (w * xT)[H, T]   (relu)
  yT[H, T]   = up[D, H]^T @ hidT[D, T]

The routing weight is folded into the gathered activations on the
host: w >= 0 (uniform routing weights times a 0/1 attention mask) and
relu is positively homogeneous, so relu((w*x) @ down) @ up =
w * (relu(x @ down) @ up). That removes the on-device scale pass and
lets both matmuls stream exactly T real token columns with all-full
128-row output tiles.
"""

import os
import sys
import time

import numpy as np

for _p in ("/opt/trn_rl_repo", "/root/.axon_site/_ro/trn_rl_repo"):
    if os.path.isdir(_p) and _p not in sys.path:
        sys.path.append(_p)

import ml_dtypes

import concourse.bass as bass
import concourse.mybir as mybir
from concourse import bacc
from concourse.bass_utils import run_bass_kernel_spmd
from concourse.tile import TileContext

BF16 = ml_dtypes.bfloat16

B, S, H, E, K, D = 1, 4096, 1024, 8, 2, 2048
N = B * S
P = 128
KH = H // P   # 8 contraction tiles for the down matmul
KD = D // P   # 16 contraction tiles for the up matmul
NCORES = 8

# Exposed for test harness introspection (exec_time_ns etc).
LAST_RESULT = None


def _chunks(total: int, maxc: int = 512) -> list[tuple[int, int]]:
    """Equal-ish (offset, size) split of `total` into ceil(total/maxc)
    pieces — keeps every matmul moving-dim well above the dispatch
    floor instead of leaving a tiny remainder chunk."""
    n = -(-total // maxc)
    base, rem = divmod(total, n)
    out, off = [], 0
    for i in range(n):
        sz = base + (1 if i < rem else 0)
        out.append((off, sz))
        off += sz
    return out


def _build_bass(t_cap: int) -> bass.Bass:
    """One expert's MLP: yT[H,T] = up^T @ relu(down^T @ xT)."""
    bf16 = mybir.dt.bfloat16
    f32 = mybir.dt.float32

    nc = bacc.Bacc()
    xT = nc.dram_tensor("xT", [H, t_cap], bf16, kind="ExternalInput")
    dw = nc.dram_tensor("dw", [H, D], bf16, kind="ExternalInput")
    up = nc.dram_tensor("up", [D, H], bf16, kind="ExternalInput")
    yT = nc.dram_tensor("yT", [H, t_cap], f32, kind="ExternalOutput")

    with TileContext(nc) as tc:
        with (
            tc.tile_pool(name="const", bufs=1) as const,
            tc.tile_pool(name="psum", bufs=1, space="PSUM") as psum,
            tc.tile_pool(name="outp", bufs=4) as outp,
        ):
            dw_sb = const.tile([P, KH, D], bf16)
            xT_sb = const.tile([P, KH, t_cap], bf16)
            up_sb = const.tile([P, KD, H], bf16)
            hid_sb = const.tile([P, KD, t_cap], bf16)

            # Per-chunk loads, interleaved so contraction chunk k of both
            # mm1 operands lands together: the k-outer matmul loop below
            # can start as soon as chunk 0 arrives instead of waiting for
            # the full 6.5MB. dw is further split into column halves so
            # the first half of the D tiles (mh=0 groups) can run while
            # the second half is still in flight. up goes on the same
            # ring strictly after dw so it doesn't steal HBM bandwidth
            # from the critical path; xT streams in parallel on the ACT
            # ring.
            hD = D // 2
            tchunks = _chunks(t_cap)
            n0_off, n0 = tchunks[0]
            rings = [nc.sync, nc.scalar]
            # Phase 1 (critical path): weight half A + the first token
            # chunk, alternating rings per k so chunk k of both operands
            # lands at PE consumption rate.
            for k in range(KH):
                if k == 0:
                    # Finest granularity for the very first dependencies:
                    # matmuls (k=0, m=0..3) need only the first dw column
                    # quarter-chunk, so split k=0's transfers in half.
                    qD = hD // 2
                    nc.sync.dma_start(dw_sb[:, 0, :qD], dw[:P, :qD])
                    nc.scalar.dma_start(
                        xT_sb[:, 0, :n0], xT[:P, :n0]
                    )
                    nc.sync.dma_start(dw_sb[:, 0, qD:hD], dw[:P, qD:hD])
                    continue
                rings[k % 2].dma_start(
                    dw_sb[:, k, :hD], dw[k * P : (k + 1) * P, :hD]
                )
                rings[1 - k % 2].dma_start(
                    xT_sb[:, k, :n0], xT[k * P : (k + 1) * P, :n0]
                )
            # Phase 2: weight half B (needed from ~21us).
            for k in range(KH):
                rings[k % 2].dma_start(
                    dw_sb[:, k, hD:], dw[k * P : (k + 1) * P, hD:]
                )
            # Phase 3: remaining token chunks.
            for off, sz in tchunks[1:]:
                for k in range(KH):
                    rings[k % 2].dma_start(
                        xT_sb[:, k, off : off + sz],
                        xT[k * P : (k + 1) * P, off : off + sz],
                    )
            # Phase 4: up weights (needed only when mm2 starts ~75us).
            for k in range(KD):
                rings[k % 2].dma_start(up_sb[:, k, :], up[k * P : (k + 1) * P, :])

            # Warm up the PE clock (HAM un-throttles after ~3.4us of
            # sustained activity) with dummy matmuls that depend on
            # nothing but a memset, so the real matmuls below run at
            # 2.4GHz from the start instead of 1.2GHz.
            warm_sb = const.tile([P, 640], bf16)
            nc.vector.memset(warm_sb[:], 0.0)
            warm_ps = psum.tile([P, 512], f32, tag="ps0", name="warm_ps")
            for i in range(10):
                nc.tensor.matmul(
                    warm_ps[:],
                    warm_sb[:, :P],
                    warm_sb[:, P:640],
                    start=(i == 0),
                    stop=(i == 9),
                )

            # mm1: hidT[D, T] = down^T @ xT with relu, k-outermost over 8
            # concurrent PSUM accumulation groups so each matmul only
            # depends on input chunk k.
            for n_off, n_size in tchunks:
                for mh in range(KD // 8):
                    pss = [
                        psum.tile([P, n_size], f32, tag=f"ps{m}", name=f"ps{m}")
                        for m in range(8)
                    ]
                    for k in range(KH):
                        for m in range(8):
                            md = mh * 8 + m
                            nc.tensor.matmul(
                                pss[m][:],
                                dw_sb[:, k, md * P : (md + 1) * P],
                                xT_sb[:, k, n_off : n_off + n_size],
                                start=(k == 0),
                                stop=(k == KH - 1),
                            )
                    for m in range(8):
                        md = mh * 8 + m
                        nc.vector.tensor_scalar_max(
                            hid_sb[:, md, n_off : n_off + n_size], pss[m][:], 0.0
                        )

            # mm2: yT[H, T] = up^T @ hidT. M runs over H (8 full tiles),
            # the moving dim streams exactly the real token columns.
            gi = 0
            for mh in range(H // P):
                for n_off, n_size in tchunks:
                    ps = psum.tile([P, n_size], f32, tag=f"ps{gi % 8}")
                    gi += 1
                    for k in range(KD):
                        nc.tensor.matmul(
                            ps[:],
                            up_sb[:, k, mh * P : (mh + 1) * P],
                            hid_sb[:, k, n_off : n_off + n_size],
                            start=(k == 0),
                            stop=(k == KD - 1),
                        )
                    yt = outp.tile([P, n_size], f32, tag="yt")
                    last = mh == H // P - 1 and n_off + n_size >= t_cap
                    if last:
                        # Split the final store so the kernel-tail wait is
                        # on a half-size transfer.
                        hs = n_size // 2
                        for a, b in ((0, hs), (hs, n_size)):
                            nc.vector.tensor_copy(yt[:, a:b], ps[:, a:b])
                            nc.sync.dma_start(
                                yT[mh * P : (mh + 1) * P, n_off + a : n_off + b],
                                yt[:, a:b],
                            )
                    else:
                        nc.vector.tensor_copy(yt[:], ps[:])
                        nc.sync.dma_start(
                            yT[mh * P : (mh + 1) * P, n_off : n_off + n_size], yt[:]
                        )
    nc.compile()
    return nc


def _route(expert_weights, chosen_expert_indices, attention_mask):
    """Host-side routing. Returns (token ids per expert, weights per
    expert, padded positions per (token, k) pair, T_cap)."""
    idx = np.asarray(chosen_expert_indices).reshape(N, K).astype(np.int64)
    wts = np.asarray(expert_weights).reshape(N, K).astype(np.float32)
    mask = np.asarray(attention_mask).reshape(N, 1).astype(np.float32)
    wts = wts * mask

    flat_e = idx.reshape(-1)  # [N*K]
    order = np.argsort(flat_e, kind="stable")
    counts = np.bincount(flat_e, minlength=E)
    offsets = np.zeros(E + 1, np.int64)
    np.cumsum(counts, out=offsets[1:])
    t_cap = max(P, int(counts.max()))

    rank = np.empty(N * K, np.int64)
    rank[order] = np.arange(N * K) - np.repeat(offsets[:-1], counts)
    pad_pos = flat_e * t_cap + rank  # row of pair (n,k) in concat output

    toks = [order[offsets[e] : offsets[e + 1]] // K for e in range(E)]
    w_e = [wts.reshape(-1)[order[offsets[e] : offsets[e + 1]]] for e in range(E)]
    return toks, w_e, pad_pos, t_cap


def kernel(x, attention_mask, expert_weights, chosen_expert_indices, down_proj, up_proj):
    global LAST_RESULT
    xt = np.asarray(x, dtype=np.float32).reshape(N, H)
    toks, w_e, pad_pos, t_cap = _route(
        expert_weights, chosen_expert_indices, attention_mask
    )

    xT_full = np.ascontiguousarray(xt.T)  # [H, N]
    down = np.asarray(down_proj, dtype=np.float32)
    up = np.asarray(up_proj, dtype=np.float32)

    in_maps = []
    for e in range(E):
        t_e = len(toks[e])
        xTg = np.zeros((H, t_cap), dtype=BF16)
        # routing weight folded into the activations (w >= 0, relu is
        # positively homogeneous) so the device output needs no scaling
        xTg[:, :t_e] = (xT_full[:, toks[e]] * w_e[e][None, :]).astype(BF16)
        in_maps.append(
            {"xT": xTg, "dw": down[e].astype(BF16), "up": up[e].astype(BF16)}
        )

    nc = _build_bass(t_cap)
    # First execution of a freshly loaded NEFF occasionally fails with a
    # transient NRT_EXEC_UNIT_UNRECOVERABLE; a retry has always succeeded.
    last_err = None
    for attempt in range(3):
        try:
            res = run_bass_kernel_spmd(nc, in_maps, core_ids=list(range(NCORES)))
            break
        except Exception as e:  # noqa: BLE001
            last_err = e
            time.sleep(3.0)
    else:
        raise last_err
    LAST_RESULT = res

    # res[e]["yT"] is [H, t_cap]; stack to [E*t_cap, H] token-major.
    y_all = np.concatenate(
        [np.ascontiguousarray(res.results[e]["yT"].T) for e in range(E)], axis=0
    )
    contrib = y_all[pad_pos]  # [N*K, H]
    out = xt + contrib[0::2] + contrib[1::2]
    return out.reshape(B, S, H).astype(np.float32)


# revision 41
# speedup vs baseline: 1.0091x; 1.0091x over previous
"""MoE MLP (top-2 of 8 experts) on 8 Trainium2 NeuronCores.

Strategy: expert parallelism. Each of the 8 cores owns one expert.
Host-side (inside kernel()): route tokens to experts, gather each
expert's tokens (pre-scaled by their routing weight) into a dense
padded [H, T_cap] activation block (transposed so it is directly
usable as the matmul moving operand), and ship it with that expert's
weights to its core. Each core runs two dense bf16 matmuls
(down -> relu -> up) entirely out of SBUF. Host-side combine is a
pure gather-add: every token has exactly K=2 expert contributions.

Device compute per core (bf16, fp32 PSUM accumulation):
  hidT[D, T] = down[H, D]^T @ (w * xT)[H, T]   (relu)
  yT[H, T]   = up[D, H]^T @ hidT[D, T]

The routing weight is folded into the gathered activations on the
host: w >= 0 (uniform routing weights times a 0/1 attention mask) and
relu is positively homogeneous, so relu((w*x) @ down) @ up =
w * (relu(x @ down) @ up). That removes the on-device scale pass and
lets both matmuls stream exactly T real token columns with all-full
128-row output tiles.
"""

import os
import sys
import time

import numpy as np

for _p in ("/opt/trn_rl_repo", "/root/.axon_site/_ro/trn_rl_repo"):
    if os.path.isdir(_p) and _p not in sys.path:
        sys.path.append(_p)

import ml_dtypes

import concourse.bass as bass
import concourse.mybir as mybir
from concourse import bacc
from concourse.bass_utils import run_bass_kernel_spmd
from concourse.tile import TileContext

BF16 = ml_dtypes.bfloat16

B, S, H, E, K, D = 1, 4096, 1024, 8, 2, 2048
N = B * S
P = 128
KH = H // P   # 8 contraction tiles for the down matmul
KD = D // P   # 16 contraction tiles for the up matmul
NCORES = 8

# Exposed for test harness introspection (exec_time_ns etc).
LAST_RESULT = None


def _chunks(total: int, maxc: int = 512) -> list[tuple[int, int]]:
    """Equal-ish (offset, size) split of `total` into ceil(total/maxc)
    pieces — keeps every matmul moving-dim well above the dispatch
    floor instead of leaving a tiny remainder chunk."""
    n = -(-total // maxc)
    base, rem = divmod(total, n)
    out, off = [], 0
    for i in range(n):
        sz = base + (1 if i < rem else 0)
        out.append((off, sz))
        off += sz
    return out


def _build_bass(t_cap: int) -> bass.Bass:
    """One expert's MLP: yT[H,T] = up^T @ relu(down^T @ xT)."""
    bf16 = mybir.dt.bfloat16
    f32 = mybir.dt.float32

    nc = bacc.Bacc()
    xT = nc.dram_tensor("xT", [H, t_cap], bf16, kind="ExternalInput")
    dw = nc.dram_tensor("dw", [H, D], bf16, kind="ExternalInput")
    up = nc.dram_tensor("up", [D, H], bf16, kind="ExternalInput")
    yT = nc.dram_tensor("yT", [H, t_cap], f32, kind="ExternalOutput")

    with TileContext(nc) as tc:
        with (
            tc.tile_pool(name="const", bufs=1) as const,
            tc.tile_pool(name="psum", bufs=1, space="PSUM") as psum,
            tc.tile_pool(name="outp", bufs=4) as outp,
        ):
            dw_sb = const.tile([P, KH, D], bf16)
            xT_sb = const.tile([P, KH, t_cap], bf16)
            up_sb = const.tile([P, KD, H], bf16)
            hid_sb = const.tile([P, KD, t_cap], bf16)

            # Per-chunk loads, interleaved so contraction chunk k of both
            # mm1 operands lands together: the k-outer matmul loop below
            # can start as soon as chunk 0 arrives instead of waiting for
            # the full 6.5MB. dw is further split into column halves so
            # the first half of the D tiles (mh=0 groups) can run while
            # the second half is still in flight. up goes on the same
            # ring strictly after dw so it doesn't steal HBM bandwidth
            # from the critical path; xT streams in parallel on the ACT
            # ring.
            hD = D // 2
            tchunks = _chunks(t_cap)
            n0_off, n0 = tchunks[0]
            rings = [nc.sync, nc.scalar]
            # Phase 1 (critical path): weight half A + the first token
            # chunk, alternating rings per k so chunk k of both operands
            # lands at PE consumption rate.
            for k in range(KH):
                if k == 0:
                    # Finest granularity for the very first dependencies:
                    # matmuls (k=0, m=0..3) need only the first dw column
                    # quarter-chunk, so split k=0's transfers in half.
                    qD = hD // 2
                    nc.sync.dma_start(dw_sb[:, 0, :qD], dw[:P, :qD])
                    nc.scalar.dma_start(
                        xT_sb[:, 0, :n0], xT[:P, :n0]
                    )
                    nc.sync.dma_start(dw_sb[:, 0, qD:hD], dw[:P, qD:hD])
                    continue
                rings[k % 2].dma_start(
                    dw_sb[:, k, :hD], dw[k * P : (k + 1) * P, :hD]
                )
                rings[1 - k % 2].dma_start(
                    xT_sb[:, k, :n0], xT[k * P : (k + 1) * P, :n0]
                )
                if k == 5:
                    # Pull the first two B-half chunks ahead so the
                    # second half of the D tiles never waits on them.
                    for kb in (0, 1):
                        rings[kb % 2].dma_start(
                            dw_sb[:, kb, hD:], dw[kb * P : (kb + 1) * P, hD:]
                        )
            # Phase 2: weight half B (needed from ~21us).
            for k in range(2, KH):
                rings[k % 2].dma_start(
                    dw_sb[:, k, hD:], dw[k * P : (k + 1) * P, hD:]
                )
            # Phase 3: remaining token chunks.
            for off, sz in tchunks[1:]:
                for k in range(KH):
                    rings[k % 2].dma_start(
                        xT_sb[:, k, off : off + sz],
                        xT[k * P : (k + 1) * P, off : off + sz],
                    )
            # Phase 4: up weights (needed only when mm2 starts ~75us).
            for k in range(KD):
                rings[k % 2].dma_start(up_sb[:, k, :], up[k * P : (k + 1) * P, :])

            # Warm up the PE clock (HAM un-throttles after ~3.4us of
            # sustained activity) with dummy matmuls that depend on
            # nothing but a memset, so the real matmuls below run at
            # 2.4GHz from the start instead of 1.2GHz.
            warm_sb = const.tile([P, 640], bf16)
            nc.vector.memset(warm_sb[:], 0.0)
            warm_ps = psum.tile([P, 512], f32, tag="ps0", name="warm_ps")
            for i in range(10):
                nc.tensor.matmul(
                    warm_ps[:],
                    warm_sb[:, :P],
                    warm_sb[:, P:640],
                    start=(i == 0),
                    stop=(i == 9),
                )

            # mm1: hidT[D, T] = down^T @ xT with relu, k-outermost over 8
            # concurrent PSUM accumulation groups so each matmul only
            # depends on input chunk k.
            for n_off, n_size in tchunks:
                for mh in range(KD // 8):
                    pss = [
                        psum.tile([P, n_size], f32, tag=f"ps{m}", name=f"ps{m}")
                        for m in range(8)
                    ]
                    for k in range(KH):
                        for m in range(8):
                            md = mh * 8 + m
                            nc.tensor.matmul(
                                pss[m][:],
                                dw_sb[:, k, md * P : (md + 1) * P],
                                xT_sb[:, k, n_off : n_off + n_size],
                                start=(k == 0),
                                stop=(k == KH - 1),
                            )
                    for m in range(8):
                        md = mh * 8 + m
                        nc.vector.tensor_scalar_max(
                            hid_sb[:, md, n_off : n_off + n_size], pss[m][:], 0.0
                        )

            # mm2: yT[H, T] = up^T @ hidT. M runs over H (8 full tiles),
            # the moving dim streams exactly the real token columns.
            gi = 0
            for mh in range(H // P):
                for n_off, n_size in tchunks:
                    ps = psum.tile([P, n_size], f32, tag=f"ps{gi % 8}")
                    gi += 1
                    for k in range(KD):
                        nc.tensor.matmul(
                            ps[:],
                            up_sb[:, k, mh * P : (mh + 1) * P],
                            hid_sb[:, k, n_off : n_off + n_size],
                            start=(k == 0),
                            stop=(k == KD - 1),
                        )
                    yt = outp.tile([P, n_size], f32, tag="yt")
                    nc.vector.tensor_copy(yt[:], ps[:])
                    nc.sync.dma_start(
                        yT[mh * P : (mh + 1) * P, n_off : n_off + n_size], yt[:]
                    )
    nc.compile()
    return nc


def _route(expert_weights, chosen_expert_indices, attention_mask):
    """Host-side routing. Returns (token ids per expert, weights per
    expert, padded positions per (token, k) pair, T_cap)."""
    idx = np.asarray(chosen_expert_indices).reshape(N, K).astype(np.int64)
    wts = np.asarray(expert_weights).reshape(N, K).astype(np.float32)
    mask = np.asarray(attention_mask).reshape(N, 1).astype(np.float32)
    wts = wts * mask

    flat_e = idx.reshape(-1)  # [N*K]
    order = np.argsort(flat_e, kind="stable")
    counts = np.bincount(flat_e, minlength=E)
    offsets = np.zeros(E + 1, np.int64)
    np.cumsum(counts, out=offsets[1:])
    t_cap = max(P, int(counts.max()))

    rank = np.empty(N * K, np.int64)
    rank[order] = np.arange(N * K) - np.repeat(offsets[:-1], counts)
    pad_pos = flat_e * t_cap + rank  # row of pair (n,k) in concat output

    toks = [order[offsets[e] : offsets[e + 1]] // K for e in range(E)]
    w_e = [wts.reshape(-1)[order[offsets[e] : offsets[e + 1]]] for e in range(E)]
    return toks, w_e, pad_pos, t_cap


def kernel(x, attention_mask, expert_weights, chosen_expert_indices, down_proj, up_proj):
    global LAST_RESULT
    xt = np.asarray(x, dtype=np.float32).reshape(N, H)
    toks, w_e, pad_pos, t_cap = _route(
        expert_weights, chosen_expert_indices, attention_mask
    )

    xT_full = np.ascontiguousarray(xt.T)  # [H, N]
    down = np.asarray(down_proj, dtype=np.float32)
    up = np.asarray(up_proj, dtype=np.float32)

    in_maps = []
    for e in range(E):
        t_e = len(toks[e])
        xTg = np.zeros((H, t_cap), dtype=BF16)
        # routing weight folded into the activations (w >= 0, relu is
        # positively homogeneous) so the device output needs no scaling
        xTg[:, :t_e] = (xT_full[:, toks[e]] * w_e[e][None, :]).astype(BF16)
        in_maps.append(
            {"xT": xTg, "dw": down[e].astype(BF16), "up": up[e].astype(BF16)}
        )

    nc = _build_bass(t_cap)
    # First execution of a freshly loaded NEFF occasionally fails with a
    # transient NRT_EXEC_UNIT_UNRECOVERABLE; a retry has always succeeded.
    last_err = None
    for attempt in range(3):
        try:
            res = run_bass_kernel_spmd(nc, in_maps, core_ids=list(range(NCORES)))
            break
        except Exception as e:  # noqa: BLE001
            last_err = e
            time.sleep(3.0)
    else:
        raise last_err
    LAST_RESULT = res

    # res[e]["yT"] is [H, t_cap]; stack to [E*t_cap, H] token-major.
    y_all = np.concatenate(
        [np.ascontiguousarray(res.results[e]["yT"].T) for e in range(E)], axis=0
    )
    contrib = y_all[pad_pos]  # [N*K, H]
    out = xt + contrib[0::2] + contrib[1::2]
    return out.reshape(B, S, H).astype(np.float32)
